# revision 8
# baseline (speedup 1.0000x reference)
"""Causal self-attention (B=4, T=2048, C=1024, H=16) on 8 Trainium2 NeuronCores.

Sharding (per the hint): data-parallel over batch (4) x tensor-parallel over
head halves (2) = 8 cores. Core c handles batch b = c//2 and heads
[8*(c%2), 8*(c%2)+8). Each core computes:
  - qkv projection for its 8 heads from x[b]^T (transposed on host)
  - causal attention in a fully transposed layout:
      scores^T[key, q] = k_chunk @ q^T   (no on-chip transposes anywhere)
      probs^T = exp(scale * scores^T) * causal_mask
      out^T[d, q]  accumulated as v_aug^T @ probs^T, where v_aug has a ones
      column so row 64 of the accumulator is the softmax denominator
  - partial out-projection with its 512-row slice of w_out
Host sums the two partial outputs per batch element (the tensor-parallel
all-reduce done on host, since the output must be gathered anyway).

All matmul operands use dtype float32r (fp32 bits, PE streams them at 1
row/cycle instead of 4; inputs are rounded by the producing engine,
accumulation stays fp32 in PSUM).
"""
import sys

if "/opt/trn_rl_repo" not in sys.path:
    sys.path.insert(0, "/opt/trn_rl_repo")

import numpy as np

T = 2048
C = 1024
HLOC = 8          # heads per core
DK = 64
HD = HLOC * DK    # 512 local head dims
KC = C // 128     # 8 contraction chunks for the qkv projection
NMT = HD // 128   # 4 tiles of q^T / k^T rows
NVT = T // 128    # 16 v tiles
NQT = T // 512    # 4 q tiles of 512
SCALE = DK ** -0.5

_CACHE = {}


def _build_nc():
    import concourse.mybir as mybir
    import concourse.tile as tile
    from concourse import bacc
    from concourse.masks import make_upper_triangular

    F32 = mybir.dt.float32
    F32R = mybir.dt.float32r
    AF = mybir.ActivationFunctionType

    nc = bacc.Bacc("TRN2", target_bir_lowering=False, debug=False, num_devices=8)
    xT = nc.dram_tensor("xT", [C, T], F32R, kind="ExternalInput")
    wq = nc.dram_tensor("wq", [C, HD], F32R, kind="ExternalInput")
    wk = nc.dram_tensor("wk", [C, HD], F32R, kind="ExternalInput")
    wv = nc.dram_tensor("wv", [C, HD], F32R, kind="ExternalInput")
    wo = nc.dram_tensor("wo", [HD, C], F32R, kind="ExternalInput")
    y = nc.dram_tensor("y", [T, C], F32, kind="ExternalOutput")

    with tile.TileContext(nc) as tc:
        with tc.tile_pool(name="const", bufs=1) as const, \
             tc.tile_pool(name="qkv", bufs=1) as qkv, \
             tc.tile_pool(name="probsp", bufs=2) as probsp, \
             tc.tile_pool(name="drp", bufs=1) as drp, \
             tc.tile_pool(name="bsp", bufs=1) as bsp, \
             tc.tile_pool(name="psp", bufs=2, space="PSUM") as psp:
            # ---- constants ----
            cpack = const.tile([128, 193], F32)
            mask_f = cpack[:, 0:128]                        # 1 where key <= q
            make_upper_triangular(nc, mask_f, val=1.0, diag=True)
            ones_f = cpack[:, 128:192]
            nc.vector.memset(ones_f, 1.0)
            onecol_f = cpack[:, 192:193]
            nc.vector.memset(onecol_f, 1.0)
            ones_r = const.tile([1, 64], F32R)
            nc.vector.tensor_copy(ones_r, ones_f[0:1, :])
            mask3 = mask_f.unsqueeze(1).broadcast_to([128, 2, 128])

            # ---- qkv projection outputs, live through all phases ----
            qT_sb = qkv.tile([128, NMT, T], F32R)           # q^T: [head_dim, t]
            kT_sb = qkv.tile([128, NMT, T], F32R)
            v_sb = qkv.tile([128, NVT, HLOC * 65], F32R)    # v_aug: ones col per head

            # ---------------- phase 1: qkv projection ----------------
            with tc.tile_pool(name="xtw", bufs=1) as xtw, \
                 tc.tile_pool(name="wpool", bufs=2) as wpool:
                xT_sb = xtw.tile([128, KC, T], F32R)
                wv_sb = wpool.tile([128, KC, HD], F32R, tag="w", bufs=1)
                wk_sb = wpool.tile([128, KC, HD], F32R, tag="w2", bufs=1)
                wv_re = wv.rearrange("(kc p) n -> p kc n", p=128)
                wk_re = wk.rearrange("(kc p) n -> p kc n", p=128)
                for kc in range(KC):
                    nc.sync.dma_start(out=xT_sb[:, kc, :],
                                      in_=xT.ap()[kc * 128:(kc + 1) * 128, :])
                    nc.sync.dma_start(out=wv_sb[:, kc, :], in_=wv_re[:, kc, :])
                    nc.sync.dma_start(out=wk_sb[:, kc, :], in_=wk_re[:, kc, :])

                # v = x @ wv (stationary = x^T chunks)
                for mt in range(NVT):
                    ps = psp.tile([128, 512], F32, tag="aux", name=f"psv{mt}")
                    for kc in range(KC):
                        nc.tensor.matmul(
                            ps, xT_sb[:, kc, mt * 128:(mt + 1) * 128],
                            wv_sb[:, kc, :],
                            start=(kc == 0), stop=(kc == KC - 1))
                    vt = v_sb[:, mt, :].rearrange("p (h e) -> p h e", e=65)
                    nc.vector.tensor_copy(
                        vt[:, :, 0:64], ps.rearrange("p (h d) -> p h d", d=64))
                    nc.vector.tensor_copy(
                        vt[:, :, 64:65], onecol_f.broadcast_to([128, HLOC, 1]))

                # wq prefetches into the second slot of tag "w" during v
                wq_sb = wpool.tile([128, KC, HD], F32R, tag="w", bufs=1)
                wq_re = wq.rearrange("(kc p) n -> p kc n", p=128)
                for kc in range(KC):
                    nc.sync.dma_start(out=wq_sb[:, kc, :], in_=wq_re[:, kc, :])

                # k^T then q^T (nt ascending so attention unblocks in order)
                for w_sb, outT in ((wk_sb, kT_sb), (wq_sb, qT_sb)):
                    for nt in range(NQT):
                        for mt in range(NMT):
                            ps = psp.tile([128, 512], F32, tag="aux",
                                          name=f"psqk{nt}_{mt}")
                            for kc in range(KC):
                                nc.tensor.matmul(
                                    ps, w_sb[:, kc, mt * 128:(mt + 1) * 128],
                                    xT_sb[:, kc, nt * 512:(nt + 1) * 512],
                                    start=(kc == 0), stop=(kc == KC - 1))
                            nc.vector.tensor_copy(
                                outT[:, mt, nt * 512:(nt + 1) * 512], ps)

            # ---------------- phases 2+3: attention + out-projection ----------------
            with tc.tile_pool(name="attnp", bufs=1) as attnp, \
                 tc.tile_pool(name="wop", bufs=1) as wop, \
                 tc.tile_pool(name="yp", bufs=3) as yp:
                attn_sb = attnp.tile([128, NMT, T], F32R)   # attn^T: [c_in, t]
                wo_sb = wop.tile([128, NMT, C], F32R)
                wo_re = wo.rearrange("(kc p) n -> p kc n", p=128)
                for kc in range(NMT):
                    nc.sync.dma_start(out=wo_sb[:, kc, :], in_=wo_re[:, kc, :])

                for qt in range(NQT):
                    for mt in range(HLOC // 2):   # head pair (2mt, 2mt+1)
                        nkb = qt * 4 + 4
                        oa = [psp.tile([65, 512], F32, tag="oa",
                                       name=f"oa{qt}_{mt}_{s}") for s in range(2)]
                        for kb in range(nkb):
                            kbl = kb - qt * 4
                            c0 = max(kbl, 0) * 128
                            sc = psp.tile([128, 2, 512], F32, tag="sc")
                            for s in range(2):
                                po = s * 64
                                nc.tensor.matmul(
                                    sc[:, s, c0:512],
                                    kT_sb[po:po + 64, mt, kb * 128:(kb + 1) * 128],
                                    qT_sb[po:po + 64, mt, qt * 512 + c0:(qt + 1) * 512],
                                    start=True, stop=True)
                            pr = probsp.tile([128, 2, 512], F32R, tag="pr")
                            nc.scalar.activation(pr[:, :, c0:512], sc[:, :, c0:512],
                                                 AF.Exp, scale=SCALE)
                            if kbl >= 0:
                                nc.vector.tensor_mul(
                                    pr[:, :, c0:c0 + 128], pr[:, :, c0:c0 + 128],
                                    mask3)
                            for s in range(2):
                                h = 2 * mt + s
                                nc.tensor.matmul(
                                    oa[s][:, c0:512],
                                    v_sb[:, kb, h * 65:(h + 1) * 65],
                                    pr[:, s, c0:512],
                                    start=(kb == 0), stop=(kb == nkb - 1))
                        # normalize: attn^T = out^T * (1/denominator)
                        for s in range(2):
                            po = s * 64
                            dr = drp.tile([1, 512], F32R, tag="dr")
                            with nc.allow_low_precision(reason="f32r softmax denom"):
                                nc.vector.reciprocal(dr, oa[s][64:65, :])
                            bc = psp.tile([64, 512], F32, tag="aux",
                                          name=f"bc{qt}{mt}{s}")
                            nc.tensor.matmul(bc, ones_r, dr, start=True, stop=True)
                            bs = bsp.tile([64, 512], F32, tag="bs")
                            nc.vector.tensor_copy(bs, bc)
                            nc.vector.tensor_mul(
                                attn_sb[po:po + 64, mt, qt * 512:(qt + 1) * 512],
                                oa[s][0:64, :], bs)
                    # out-projection for the 4 row-tiles of this qt
                    for mtl in range(4):
                        mt3 = qt * 4 + mtl
                        yt = yp.tile([128, C], F32, tag="y")
                        for ntp in range(2):
                            ps = psp.tile([128, 512], F32, tag="aux",
                                          name=f"psy{mt3}_{ntp}")
                            for kc in range(NMT):
                                nc.tensor.matmul(
                                    ps, attn_sb[:, kc, mt3 * 128:(mt3 + 1) * 128],
                                    wo_sb[:, kc, ntp * 512:(ntp + 1) * 512],
                                    start=(kc == 0), stop=(kc == NMT - 1))
                            nc.vector.tensor_copy(yt[:, ntp * 512:(ntp + 1) * 512], ps)
                        nc.sync.dma_start(out=y.ap()[mt3 * 128:(mt3 + 1) * 128, :],
                                          in_=yt)
    nc.compile()
    return nc


def _shard_inputs(x, w_qkv, w_out):
    in_maps = []
    for c in range(8):
        b, hh = c // 2, c % 2
        cols = slice(hh * HD, (hh + 1) * HD)
        in_maps.append({
            "xT": np.ascontiguousarray(x[b].T),
            "wq": np.ascontiguousarray(w_qkv[:, 0 * C:1 * C][:, cols]),
            "wk": np.ascontiguousarray(w_qkv[:, 1 * C:2 * C][:, cols]),
            "wv": np.ascontiguousarray(w_qkv[:, 2 * C:3 * C][:, cols]),
            "wo": np.ascontiguousarray(w_out[hh * HD:(hh + 1) * HD, :]),
        })
    return in_maps


def kernel(x, w_qkv, w_out):
    from concourse.bass_utils import run_bass_kernel_spmd

    x = np.asarray(x, dtype=np.float32)
    w_qkv = np.asarray(w_qkv, dtype=np.float32)
    w_out = np.asarray(w_out, dtype=np.float32)

    if "nc" not in _CACHE:
        _CACHE["nc"] = _build_nc()
    nc = _CACHE["nc"]

    in_maps = _shard_inputs(x, w_qkv, w_out)
    res = run_bass_kernel_spmd(nc, in_maps, core_ids=list(range(8)))
    outs = [res.results[c]["y"] for c in range(8)]
    out = np.stack([outs[2 * b] + outs[2 * b + 1] for b in range(4)])
    return out.astype(np.float32)


# revision 15
# speedup vs baseline: 1.2283x; 1.2283x over previous
"""Causal self-attention (B=4, T=2048, C=1024, H=16) on 8 Trainium2 NeuronCores.

Sharding (per the hint): data-parallel over batch (4) x tensor-parallel over
head halves (2) = 8 cores. Core c handles batch b = c//2 and heads
[8*(c%2), 8*(c%2)+8). Each core computes:
  - qkv projection for its 8 heads from x[b]^T (transposed on host)
  - causal attention in a fully transposed layout:
      scores^T[key, q] = k_chunk @ q^T   (no on-chip transposes anywhere)
      probs^T = exp(scale * scores^T) * causal_mask
      out^T[d, q]  accumulated as v_aug^T @ probs^T, where v_aug has a ones
      column so row 64 of the accumulator is the softmax denominator
  - partial out-projection with its 512-row slice of w_out
Host sums the two partial outputs per batch element (the tensor-parallel
all-reduce done on host, since the output must be gathered anyway).

Attention-path matmul operands use dtype float32r (fp32 bits, PE streams them
at 1 row/cycle instead of 4; rounded by the producing engine, fp32 PSUM
accumulation). The qkv projection inputs (x^T and w_q/k/v) are bf16 when
PROJ_BF16 is on, which halves their DMA time and SBUF footprint — that's what
lets every pool co-reside so the projection, attention, and out-projection all
overlap in one software-pipelined stream.

Emission is interleaved per 512-wide q-block: v tiles, k^T(nt), q^T(nt), then
attention qt=nt (with the previous block's out-projection rows slotted
between head pairs), so the scalar engine's exp stream — the phase-2
bottleneck — starts as early as possible and never starves.
"""
import sys

if "/opt/trn_rl_repo" not in sys.path:
    sys.path.insert(0, "/opt/trn_rl_repo")

import numpy as np

T = 2048
C = 1024
HLOC = 8          # heads per core
DK = 64
HD = HLOC * DK    # 512 local head dims
KC = C // 128     # 8 contraction chunks for the qkv projection
NMT = HD // 128   # 4 tiles of q^T / k^T rows
NVT = T // 128    # 16 v tiles
NQT = T // 512    # 4 q tiles of 512
SCALE = DK ** -0.5

PROJ_BF16 = True  # bf16 inputs for the qkv projection (x^T, w_q/k/v)

_CACHE = {}


def _build_nc(probs_bufs=3, proj_bf16=PROJ_BF16, interleave=True):
    import concourse.mybir as mybir
    import concourse.tile as tile
    from concourse import bacc
    from concourse.masks import make_upper_triangular

    F32 = mybir.dt.float32
    F32R = mybir.dt.float32r
    BF16 = mybir.dt.bfloat16
    AF = mybir.ActivationFunctionType
    in_dt = BF16 if proj_bf16 else F32R

    nc = bacc.Bacc("TRN2", target_bir_lowering=False, debug=False, num_devices=8)
    xT = nc.dram_tensor("xT", [C, T], in_dt, kind="ExternalInput")
    wq = nc.dram_tensor("wq", [C, HD], in_dt, kind="ExternalInput")
    wk = nc.dram_tensor("wk", [C, HD], in_dt, kind="ExternalInput")
    wv = nc.dram_tensor("wv", [C, HD], in_dt, kind="ExternalInput")
    wo = nc.dram_tensor("wo", [HD, C], F32R, kind="ExternalInput")
    y = nc.dram_tensor("y", [T, C], F32, kind="ExternalOutput")

    with tile.TileContext(nc) as tc:
        with tc.tile_pool(name="const", bufs=1) as const, \
             tc.tile_pool(name="qkv", bufs=1) as qkv, \
             tc.tile_pool(name="qTp", bufs=2) as qTp, \
             tc.tile_pool(name="xtw", bufs=1) as xtw, \
             tc.tile_pool(name="wpool", bufs=1) as wpool, \
             tc.tile_pool(name="attnp", bufs=2) as attnp, \
             tc.tile_pool(name="probsp", bufs=probs_bufs) as probsp, \
             tc.tile_pool(name="drp", bufs=2) as drp, \
             tc.tile_pool(name="bsp", bufs=2) as bsp, \
             tc.tile_pool(name="wop", bufs=1) as wop, \
             tc.tile_pool(name="yp", bufs=2) as yp, \
             tc.tile_pool(name="psp", bufs=2, space="PSUM") as psp:
            # ---- constants ----
            cpack = const.tile([128, 193], F32)
            mask_f = cpack[:, 0:128]                        # 1 where key <= q
            make_upper_triangular(nc, mask_f, val=1.0, diag=True)
            ones_f = cpack[:, 128:192]
            nc.vector.memset(ones_f, 1.0)
            onecol_f = cpack[:, 192:193]
            nc.vector.memset(onecol_f, 1.0)
            ones_r = const.tile([1, 64], F32R)
            nc.vector.tensor_copy(ones_r, ones_f[0:1, :])
            mask3 = mask_f.unsqueeze(1).broadcast_to([128, 2, 128])

            # ---- long-lived tiles ----
            kT_sb = qkv.tile([128, NMT, T], F32R)           # k^T: [head_dim, t]
            v_sb = qkv.tile([128, NVT, HLOC * 65], F32R)    # v_aug: ones col per head
            xT_sb = xtw.tile([128, KC, T], in_dt)
            wo_sb = wop.tile([128, NMT, C], F32R)

            wv_sb = wpool.tile([128, KC, HD], in_dt, tag="w")
            wk_sb = wpool.tile([128, KC, HD], in_dt, tag="w2")
            wv_re = wv.rearrange("(kc p) n -> p kc n", p=128)
            wk_re = wk.rearrange("(kc p) n -> p kc n", p=128)
            for kc in range(KC):
                nc.sync.dma_start(out=xT_sb[:, kc, :],
                                  in_=xT.ap()[kc * 128:(kc + 1) * 128, :])
                nc.sync.dma_start(out=wv_sb[:, kc, :], in_=wv_re[:, kc, :])
                nc.sync.dma_start(out=wk_sb[:, kc, :], in_=wk_re[:, kc, :])
            wo_re = wo.rearrange("(kc p) n -> p kc n", p=128)
            for kc in range(NMT):
                nc.sync.dma_start(out=wo_sb[:, kc, :], in_=wo_re[:, kc, :])

            ph1_tags = ["aux", "oa", "sc"]
            pscnt = [0]

            def ph1_psum(name):
                t = psp.tile([128, 512], F32, tag=ph1_tags[pscnt[0] % 3], name=name)
                pscnt[0] += 1
                return t

            def emit_v_tile(mt):
                ps = ph1_psum(f"psv{mt}")
                for kc in range(KC):
                    nc.tensor.matmul(
                        ps, xT_sb[:, kc, mt * 128:(mt + 1) * 128],
                        wv_sb[:, kc, :],
                        start=(kc == 0), stop=(kc == KC - 1))
                vt = v_sb[:, mt, :].rearrange("p (h e) -> p h e", e=65)
                nc.vector.tensor_copy(
                    vt[:, :, 0:64], ps.rearrange("p (h d) -> p h d", d=64))
                nc.vector.tensor_copy(
                    vt[:, :, 64:65], onecol_f.broadcast_to([128, HLOC, 1]))

            def emit_qk_block(w_sb, outT, nt, out_col0):
                # outT columns [out_col0, out_col0+512) over all 4 row tiles
                for mt in range(NMT):
                    ps = ph1_psum(f"psqk{outT.name}_{nt}_{mt}")
                    for kc in range(KC):
                        nc.tensor.matmul(
                            ps, w_sb[:, kc, mt * 128:(mt + 1) * 128],
                            xT_sb[:, kc, nt * 512:(nt + 1) * 512],
                            start=(kc == 0), stop=(kc == KC - 1))
                    nc.vector.tensor_copy(
                        outT[:, mt, out_col0:out_col0 + 512], ps)

            def emit_attention_pair(qt, mt, qT_t, attn_t):
                # head pair (2mt, 2mt+1) for q columns [qt*512, (qt+1)*512)
                nkb = qt * 4 + 4
                oa = [psp.tile([65, 512], F32, tag="oa",
                               name=f"oa{qt}_{mt}_{s}") for s in range(2)]
                for kb in range(nkb):
                    kbl = kb - qt * 4
                    c0 = max(kbl, 0) * 128
                    sc = psp.tile([128, 2, 512], F32, tag="sc")
                    for s in range(2):
                        po = s * 64
                        nc.tensor.matmul(
                            sc[:, s, c0:512],
                            kT_sb[po:po + 64, mt, kb * 128:(kb + 1) * 128],
                            qT_t[po:po + 64, mt, c0:512],
                            start=True, stop=True)
                    pr = probsp.tile([128, 2, 512], F32R, tag="pr")
                    nc.scalar.activation(pr[:, :, c0:512], sc[:, :, c0:512],
                                         AF.Exp, scale=SCALE)
                    if kbl >= 0:
                        nc.vector.tensor_mul(
                            pr[:, :, c0:c0 + 128], pr[:, :, c0:c0 + 128], mask3)
                    for s in range(2):
                        h = 2 * mt + s
                        nc.tensor.matmul(
                            oa[s][:, c0:512],
                            v_sb[:, kb, h * 65:(h + 1) * 65],
                            pr[:, s, c0:512],
                            start=(kb == 0), stop=(kb == nkb - 1))
                # normalize: attn^T = out^T * (1/denominator)
                for s in range(2):
                    po = s * 64
                    dr = drp.tile([1, 512], F32R, tag="dr")
                    with nc.allow_low_precision(reason="f32r softmax denom"):
                        nc.vector.reciprocal(dr, oa[s][64:65, :])
                    bc = psp.tile([64, 512], F32, tag="aux", name=f"bc{qt}{mt}{s}")
                    nc.tensor.matmul(bc, ones_r, dr, start=True, stop=True)
                    bs = bsp.tile([64, 512], F32, tag="bs")
                    nc.vector.tensor_copy(bs, bc)
                    nc.vector.tensor_mul(attn_t[po:po + 64, mt, :],
                                         oa[s][0:64, :], bs)

            def emit_proj_row(attn_t, mt3):
                # out-projection of y rows [mt3*128, (mt3+1)*128)
                yt = yp.tile([128, C], F32, tag="y", name=f"yt{mt3}")
                for ntp in range(2):
                    ps = psp.tile([128, 512], F32, tag="aux", name=f"psy{mt3}_{ntp}")
                    for kc in range(NMT):
                        nc.tensor.matmul(
                            ps, attn_t[:, kc, (mt3 % 4) * 128:(mt3 % 4 + 1) * 128],
                            wo_sb[:, kc, ntp * 512:(ntp + 1) * 512],
                            start=(kc == 0), stop=(kc == NMT - 1))
                    nc.vector.tensor_copy(yt[:, ntp * 512:(ntp + 1) * 512], ps)
                nc.sync.dma_start(out=y.ap()[mt3 * 128:(mt3 + 1) * 128, :], in_=yt)

            # ---------------- pipelined emission ----------------
            attn_prev = None
            wq_emitted = False
            for nt in range(NQT):
                for mtl in range(4):
                    emit_v_tile(nt * 4 + mtl)
                emit_qk_block(wk_sb, kT_sb, nt, nt * 512)
                if not wq_emitted:
                    # wq shares the wv slot; its DMA overlaps k^T(0)
                    wq_sb = wpool.tile([128, KC, HD], in_dt, tag="w3")
                    wq_re = wq.rearrange("(kc p) n -> p kc n", p=128)
                    for kc in range(KC):
                        nc.sync.dma_start(out=wq_sb[:, kc, :], in_=wq_re[:, kc, :])
                    wq_emitted = True
                qT_t = qTp.tile([128, NMT, 512], F32R, tag="qT", name=f"qT{nt}")
                emit_qk_block(wq_sb, qT_t, nt, 0)

                qt = nt
                attn_t = attnp.tile([128, NMT, 512], F32R, tag="attn",
                                    name=f"attn{qt}")
                for mt in range(HLOC // 2):
                    # previous q-block's projection rows fill PE bubbles here
                    if attn_prev is not None:
                        emit_proj_row(attn_prev, (qt - 1) * 4 + mt)
                    emit_attention_pair(qt, mt, qT_t, attn_t)
                attn_prev = attn_t
            for mtl in range(4):
                emit_proj_row(attn_prev, 3 * 4 + mtl)
    nc.compile()
    return nc


def _shard_inputs(x, w_qkv, w_out):
    if PROJ_BF16:
        import ml_dtypes
        cast = lambda a: np.ascontiguousarray(a).astype(ml_dtypes.bfloat16)
    else:
        cast = np.ascontiguousarray
    in_maps = []
    for c in range(8):
        b, hh = c // 2, c % 2
        cols = slice(hh * HD, (hh + 1) * HD)
        in_maps.append({
            "xT": cast(x[b].T),
            "wq": cast(w_qkv[:, 0 * C:1 * C][:, cols]),
            "wk": cast(w_qkv[:, 1 * C:2 * C][:, cols]),
            "wv": cast(w_qkv[:, 2 * C:3 * C][:, cols]),
            "wo": np.ascontiguousarray(w_out[hh * HD:(hh + 1) * HD, :]),
        })
    return in_maps


def kernel(x, w_qkv, w_out):
    from concourse.bass_utils import run_bass_kernel_spmd

    x = np.asarray(x, dtype=np.float32)
    w_qkv = np.asarray(w_qkv, dtype=np.float32)
    w_out = np.asarray(w_out, dtype=np.float32)

    if "nc" not in _CACHE:
        _CACHE["nc"] = _build_nc()
    nc = _CACHE["nc"]

    in_maps = _shard_inputs(x, w_qkv, w_out)
    res = run_bass_kernel_spmd(nc, in_maps, core_ids=list(range(8)))
    outs = [res.results[c]["y"] for c in range(8)]
    out = np.stack([outs[2 * b] + outs[2 * b + 1] for b in range(4)])
    return out.astype(np.float32)
